# revision 10
# baseline (speedup 1.0000x reference)
"""GAT layer kernel for Trainium2 (8 NeuronCores, Bass/Tile).

Problem: h = input_h @ W + bias; per-edge e = leakyrelu(wh1[row] + wh2[col]);
segment softmax over each destination row's 16 edges; out = segment_sum of
attn * h[col].

Strategy: destination rows sharded across 8 cores (12500 each). Each core
computes h/wh1/wh2 for its own rows into a packed fp16 table (4 nodes per
768B row: per-node 96 fp16 = 64 feats + wh2 f32 bitcast), all-gathers it,
then fetches h[col] rows per edge with the GpSimd `dma_gather` ucode
(int16 idx = col>>2, 708B fetch covering all 4 node slots).

Layout: DEST-ALIGNED — idx position j = slot*128 + dest, so dest d's 16
edges land along the free dim of partition d (slot = block_local*16 + k).
Softmax is then pure per-partition work (Act exp + free-axis accumulator
for denominators, wh1 read straight from an SBUF-resident [128, NBLK]
tile), and aggregation+slot-selection run as 4 per-slot DVE products
(one-hot masks folded into fp16 attention weights) + halving tree-reduce.
Blocks are processed in supers of 7 (98 = 14*7) to amortize op overheads.

Host-side prep is limited to slicing/layout of the *index* input (wrapped
int16 col>>2 lists, col&3 one-hot masks). All numerical work on h/W/a/bias
happens on device.
"""

import sys
import types

import numpy as np

sys.path.insert(0, "/opt/trn_rl_repo")

# ---------------------------------------------------------------- constants
N = 100000
DEG = 16
E = N * DEG
IN_F = 128
OUT_F = 64
ALPHA = 0.2
EPS = 1e-12

NCORES = 8
NLOC = N // NCORES              # 12500 dest rows per core
P = 128
NBLK = (NLOC + P - 1) // P      # 98 blocks of 128 dests
LAST_VALID = NLOC - (NBLK - 1) * P  # 84 dests in last block

PACK = 4                        # nodes per table row
NPITCH = 96                     # fp16 elems per node slot (192B)
RSTRIDE = PACK * NPITCH         # 384 fp16 = 768B row stride
FETCH = (PACK - 1) * NPITCH + 66  # 354 fp16 = 708B fetched per edge
TROWS = N // PACK               # 25000 table rows
TROWS_LOC = NLOC // PACK        # 3125 local table rows

SUPER = 4                       # blocks per phase-B iteration
SUPERS = [4] * 24 + [2]         # 24*4 + 2 = 98 blocks
SE = SUPER * DEG                # 64 slots per partition per (full) super
IWTOT = NBLK * DEG * P // 16    # 12544 int16 idx columns total


def _install_ntff_shim():
    if "antenv.axon_hooks" in sys.modules:
        return
    try:
        from trn_agent_boot.trn_boot import _ntff_profile_via_ctypes

        hook = _ntff_profile_via_ctypes("/opt/axon/libaxon_pjrt.so")
    except Exception:
        hook = None
    mod = types.ModuleType("antenv.axon_hooks")
    mod.get_axon_ntff_profile_hook = lambda: hook
    mod.set_axon_ntff_profile_hook = lambda h: None
    sys.modules["antenv.axon_hooks"] = mod


def _install_dma_gather_patch():
    """Relax bass's elem_size%256 assert (ucode needs it only for transpose)."""
    import inspect
    import textwrap

    import concourse.bass as bass

    if getattr(bass.BassGpSimd.dma_gather, "_gat_patched", False):
        return
    src = textwrap.dedent(inspect.getsource(bass.BassGpSimd.dma_gather))
    old = """    assert (
        elem_size_bytes > 0 and elem_size_bytes % 256 == 0
    )  # transpose restriction"""
    new = """    assert elem_size_bytes > 0
    if transpose:
        assert elem_size_bytes % 256 == 0"""
    assert old in src, "dma_gather source changed; patch needs updating"
    src = src.replace(old, new)
    g = dict(bass.__dict__)
    exec(src, g)
    g["dma_gather"]._gat_patched = True
    bass.BassGpSimd.dma_gather = g["dma_gather"]


# ---------------------------------------------------------------- program
_PROGRAM_CACHE = {}


def build_program():
    _install_ntff_shim()
    _install_dma_gather_patch()
    import concourse.bacc as bacc
    import concourse.bass as bass
    import concourse.tile as tile
    from concourse import mybir

    f32 = mybir.dt.float32
    f16 = mybir.dt.float16
    i16 = mybir.dt.int16

    nc = bacc.Bacc(
        "TRN2",
        num_devices=NCORES,
        num_swdge_queues=4,
        detect_race_conditions=False,
    )

    # per-core external inputs
    inT = nc.dram_tensor("inT", [IN_F, NLOC], f32, kind="ExternalInput")
    W_in = nc.dram_tensor("W_in", [IN_F, OUT_F], f32, kind="ExternalInput")
    a2_in = nc.dram_tensor("a2_in", [OUT_F, 2], f32, kind="ExternalInput")
    bias_in = nc.dram_tensor("bias_in", [OUT_F], f32, kind="ExternalInput")
    idx16w = nc.dram_tensor("idx16w", [P, IWTOT], i16, kind="ExternalInput")
    oneh_in = nc.dram_tensor("oneh_in", [P, NBLK * DEG * PACK], f16, kind="ExternalInput")

    out_d = nc.dram_tensor("out_d", [NLOC, OUT_F], f32, kind="ExternalOutput")

    with tile.TileContext(nc) as tc:
        with tc.tile_pool(name="dram", bufs=1, space="DRAM") as dpool:
            h4_loc = dpool.tile([TROWS_LOC, RSTRIDE], f16)
            h4 = dpool.tile([TROWS, RSTRIDE], f16, addr_space="Shared")

            with tc.tile_pool(name="const", bufs=1) as cpool:
                # persistent: wrapped idx + one-hot masks + wh1
                idx_sb = cpool.tile([P, IWTOT], i16)
                nc.sync.dma_start(out=idx_sb[:], in_=idx16w[:])
                oneh_sb = cpool.tile([P, NBLK * DEG * PACK], f16)
                nc.sync.dma_start(out=oneh_sb[:], in_=oneh_in[:])
                wh1_all = cpool.tile([P, NBLK], f32)

                w_sb = cpool.tile([IN_F, OUT_F], f32)
                nc.sync.dma_start(out=w_sb[:], in_=W_in[:])
                a2_sb = cpool.tile([OUT_F, 2], f32)
                nc.sync.dma_start(out=a2_sb[:], in_=a2_in[:])
                bias_col = cpool.tile([OUT_F, 1], f32)
                nc.sync.dma_start(out=bias_col[:], in_=bias_in[:, None])
                bias_rep = cpool.tile([P, OUT_F], f32)
                nc.sync.dma_start(
                    out=bias_rep[:],
                    in_=bass.AP(
                        bias_in.handle if hasattr(bias_in, "handle") else bias_in,
                        0,
                        [[0, P], [1, OUT_F]],
                    ),
                )

                # Wa2 = W @ [a_dst | a_src] (contract over OUT_F): need W^T.
                with tc.tile_pool(name="pa", bufs=1, space="PSUM") as pp0, tc.tile_pool(
                    name="sa", bufs=1
                ) as sp0:
                    idp = sp0.tile([P, P], f32)
                    from concourse.masks import make_identity

                    make_identity(nc, idp[:])
                    wt_ps = pp0.tile([P, P], f32, space="PSUM")
                    nc.tensor.transpose(out=wt_ps[:OUT_F, :IN_F], in_=w_sb[:], identity=idp[:])
                    wt_sb = sp0.tile([OUT_F, IN_F], f32)
                    nc.vector.tensor_copy(out=wt_sb[:], in_=wt_ps[:OUT_F, :IN_F])
                    wa2_ps = pp0.tile([IN_F, 2], f32, space="PSUM")
                    nc.tensor.matmul(out=wa2_ps[:], lhsT=wt_sb[:], rhs=a2_sb[:])
                    wa2_sb = cpool.tile([IN_F, 2], f32)
                    nc.vector.tensor_copy(out=wa2_sb[:], in_=wa2_ps[:])
                    w66 = cpool.tile([IN_F, OUT_F + 2], f32)
                    nc.vector.tensor_copy(out=w66[:, 0:OUT_F], in_=w_sb[:])
                    nc.vector.tensor_copy(out=w66[:, OUT_F : OUT_F + 2], in_=wa2_ps[:])
                    # ab = a2^T bias -> [2,1]; broadcast each to 128 partitions
                    ab_ps = pp0.tile([2, 1], f32, space="PSUM")
                    nc.tensor.matmul(out=ab_ps[:], lhsT=a2_sb[:], rhs=bias_col[:])
                    ab_sb = sp0.tile([2, 1], f32)
                    nc.vector.tensor_copy(out=ab_sb[:], in_=ab_ps[:])
                    ab_dram = dpool.tile([2], f32)
                    nc.sync.dma_start(
                        out=bass.AP(ab_dram[:].tensor, ab_dram[:].offset, [[1, 2], [1, 1]]),
                        in_=ab_sb[:],
                    )
                    cv2_rep = cpool.tile([P, 1], f32)
                    cv1_rep = cpool.tile([P, 1], f32)
                    nc.sync.dma_start(
                        out=cv2_rep[:],
                        in_=bass.AP(ab_dram[:].tensor, ab_dram[:].offset, [[0, P], [1, 1]]),
                    )
                    nc.sync.dma_start(
                        out=cv1_rep[:],
                        in_=bass.AP(ab_dram[:].tensor, ab_dram[:].offset + 1, [[0, P], [1, 1]]),
                    )

                # ---------------- phase A: h / wh1 / wh2 for own rows
                with tc.tile_pool(name="pha_s", bufs=1) as spA, tc.tile_pool(
                    name="pha_ps", bufs=2, space="PSUM"
                ) as ppA, tc.tile_pool(name="pha_w", bufs=2) as wpA:
                    inT_sb = spA.tile([IN_F, NLOC], f32)
                    nc.sync.dma_start(out=inT_sb[:], in_=inT[:])

                    for t in range(NBLK):
                        r0 = t * P
                        rows = min(P, NLOC - r0)
                        lhsT = inT_sb[:, r0 : r0 + rows]
                        h_ps = ppA.tile([P, OUT_F + 2], f32, tag="h_ps")
                        nc.tensor.matmul(out=h_ps[:rows, :], lhsT=lhsT, rhs=w66[:])

                        h16 = wpA.tile([P, NPITCH], f16, tag="h16")
                        nc.vector.tensor_add(
                            out=h16[:rows, 0:OUT_F],
                            in0=h_ps[:rows, 0:OUT_F],
                            in1=bias_rep[:rows, :],
                        )
                        h16f32 = h16[:].bitcast(f32)
                        nc.vector.tensor_add(
                            out=h16f32[:rows, 32:33],
                            in0=h_ps[:rows, OUT_F : OUT_F + 1],
                            in1=cv2_rep[:rows, :],
                        )
                        nc.vector.tensor_add(
                            out=wh1_all[:rows, t : t + 1],
                            in0=h_ps[:rows, OUT_F + 1 : OUT_F + 2],
                            in1=cv1_rep[:rows, :],
                        )
                        # store 128 node rows = 32 table rows at node pitch
                        nc.sync.dma_start(
                            out=bass.AP(
                                h4_loc[:].tensor,
                                h4_loc[:].offset + (r0 // PACK) * RSTRIDE,
                                [[NPITCH, rows], [1, NPITCH]],
                            ),
                            in_=h16[:rows, :],
                        )

                # ---------------- all-gather the packed table
                nc.gpsimd.collective_compute(
                    "AllGather",
                    mybir.AluOpType.bypass,
                    replica_groups=[list(range(NCORES))],
                    ins=[h4_loc.opt()],
                    outs=[h4.opt()],
                )

                # ---------------- phase B
                with tc.tile_pool(name="phb_g", bufs=2) as gp, tc.tile_pool(
                    name="phb_s", bufs=2
                ) as bp, tc.tile_pool(name="phb_m", bufs=1) as mp:
                    b0 = 0
                    for s, nb in enumerate(SUPERS):
                        se = nb * DEG              # slots this super
                        nidx = se * P
                        iw0 = b0 * DEG * P // 16   # idx column offset
                        ihw = nidx // 16
                        G = gp.tile([P, SE, FETCH], f16, tag="G")
                        ncall = nidx // 1024       # 1024-idx calls (8 slots each)
                        for half in range(ncall):
                            nc.gpsimd.dma_gather(
                                out_ap=G[:, half * 8 : (half + 1) * 8, :],
                                in_ap=h4[:, 0:FETCH],
                                idxs_ap=idx_sb[
                                    :, iw0 + half * 64 : iw0 + (half + 1) * 64
                                ],
                                num_idxs=1024,
                                num_idxs_reg=1024,
                                elem_size=FETCH,
                                elem_step=RSTRIDE,
                                queue_num=(s * ncall + half) % 4,
                            )
                        oneh = bass.AP(
                            oneh_sb[:].tensor,
                            oneh_sb[:].offset + b0 * DEG * PACK,
                            [list(oneh_sb[:].ap[0]), [PACK, se], [1, PACK]],
                        )

                        Gf32 = G[:].bitcast(f32)  # [P, SE, FETCH//2]
                        # wh2 candidates (4 slots) -> select via oneh (strided
                        # read straight out of G; compaction hop elided)
                        oneh32 = bp.tile([P, SE, PACK], f32, tag="oneh32")
                        nc.scalar.copy(
                            out=oneh32[:, 0:se, :].rearrange("p a b -> p (a b)"),
                            in_=oneh,
                        )
                        wh2s = bp.tile([P, SE, PACK], f32, tag="wh2s")
                        nc.vector.tensor_tensor(
                            out=wh2s[:, 0:se, :],
                            in0=bass.AP(
                                Gf32.tensor,
                                Gf32.offset + 32,
                                [list(Gf32.ap[0]), [FETCH // 2, se], [NPITCH // 2, PACK]],
                            ),
                            in1=oneh32[:, 0:se, :],
                            op=mybir.AluOpType.mult,
                        )
                        wh2e = bp.tile([P, SUPER, DEG], f32, tag="wh2e")
                        nc.vector.reduce_sum(
                            out=wh2e[:, 0:nb, :].rearrange("p a b -> p (a b)"),
                            in_=wh2s[:, 0:se, :],
                            axis=mybir.AxisListType.X,
                        )
                        # e0 = wh2e + wh1[dest] (wh1 broadcast along k)
                        e0 = bp.tile([P, SUPER, DEG], f32, tag="e0")
                        nc.vector.tensor_tensor(
                            out=e0[:, 0:nb, :],
                            in0=wh2e[:, 0:nb, :],
                            in1=bass.AP(
                                wh1_all[:].tensor,
                                wh1_all[:].offset + b0,
                                [list(wh1_all[:].ap[0]), [1, nb], [0, DEG]],
                            ),
                            op=mybir.AluOpType.add,
                        )
                        # leaky relu
                        esc = bp.tile([P, SE], f32, tag="esc")
                        nc.vector.tensor_scalar_mul(
                            out=esc[:, 0:se],
                            in0=e0[:, 0:nb, :].rearrange("p s k -> p (s k)"),
                            scalar1=ALPHA,
                        )
                        elr = bp.tile([P, SE], f32, tag="elr")
                        nc.vector.tensor_tensor(
                            out=elr[:, 0:se],
                            in0=e0[:, 0:nb, :].rearrange("p s k -> p (s k)"),
                            in1=esc[:, 0:se],
                            op=mybir.AluOpType.max,
                        )
                        # ex = exp(e); per-block denominators
                        ex = bp.tile([P, SUPER, DEG], f32, tag="ex")
                        nc.scalar.activation(
                            out=ex[:, 0:nb, :].rearrange("p s k -> p (s k)"),
                            in_=elr[:, 0:se],
                            func=mybir.ActivationFunctionType.Exp,
                        )
                        den = bp.tile([P, SUPER], f32, tag="den")
                        nc.vector.reduce_sum(
                            out=den[:, 0:nb], in_=ex[:, 0:nb, :], axis=mybir.AxisListType.X
                        )
                        dene = bp.tile([P, SUPER], f32, tag="dene")
                        nc.vector.tensor_scalar_add(
                            out=dene[:, 0:nb], in0=den[:, 0:nb], scalar1=EPS
                        )
                        rden = bp.tile([P, SUPER], f32, tag="rden")
                        nc.vector.reciprocal(out=rden[:, 0:nb], in_=dene[:, 0:nb])
                        # attn = ex * rden (fp16)
                        attn = bp.tile([P, SE], f16, tag="attn")
                        nc.vector.tensor_tensor(
                            out=attn[:, 0:se],
                            in0=ex[:, 0:nb, :].rearrange("p s k -> p (s k)"),
                            in1=bass.AP(
                                rden[:].tensor,
                                rden[:].offset,
                                [list(rden[:].ap[0]), [1, nb], [0, DEG]],
                            ),
                            op=mybir.AluOpType.mult,
                        )
                        # w[p, slot, m] = attn * oneh ; w8 = w expanded x8 (Act)
                        w = bp.tile([P, SE, PACK], f16, tag="w")
                        nc.vector.tensor_tensor(
                            out=w[:, 0:se, :],
                            in0=oneh,
                            in1=bass.AP(
                                attn[:].tensor,
                                attn[:].offset,
                                [list(attn[:].ap[0]), [1, se], [0, PACK]],
                            ),
                            op=mybir.AluOpType.mult,
                        )
                        w8 = bp.tile([P, SE, PACK, 8], f16, tag="w8")
                        nc.scalar.copy(
                            out=w8[:, 0:se, :, :].rearrange("p a b c -> p (a b c)"),
                            in_=bass.AP(
                                w[:].tensor,
                                w[:].offset,
                                [list(w[:].ap[0]), [1, se * PACK], [0, 8]],
                            ),
                        )
                        # per-slot products + m-tree
                        pma = mp.tile([P, SE, OUT_F], f16, tag="pma")
                        pmb = mp.tile([P, SE, OUT_F], f16, tag="pmb")
                        t01 = mp.tile([P, SE * OUT_F], f16, tag="t01")
                        t23 = mp.tile([P, SE * OUT_F], f16, tag="t23")
                        for mpair, (tdst, tsrc_a, tsrc_b) in enumerate(
                            [(t01, pma, pmb), (t23, pma, pmb)]
                        ):
                            for mm, pm in ((2 * mpair, tsrc_a), (2 * mpair + 1, tsrc_b)):
                                nc.vector.tensor_tensor(
                                    out=pm[:, 0:se, :].rearrange("p a b -> p (a b)"),
                                    in0=bass.AP(
                                        G[:].tensor,
                                        G[:].offset + mm * NPITCH,
                                        [list(G[:].ap[0]), [FETCH, se], [1, OUT_F]],
                                    ),
                                    in1=bass.AP(
                                        w8[:].tensor,
                                        w8[:].offset + mm * 8,
                                        [list(w8[:].ap[0]), [PACK * 8, se], [0, 8], [1, 8]],
                                    ),
                                    op=mybir.AluOpType.mult,
                                )
                            nc.vector.tensor_tensor(
                                out=tdst[:, 0 : se * OUT_F],
                                in0=tsrc_a[:, 0:se, :].rearrange("p a b -> p (a b)"),
                                in1=tsrc_b[:, 0:se, :].rearrange("p a b -> p (a b)"),
                                op=mybir.AluOpType.add,
                            )
                        gw = mp.tile([P, SE * OUT_F], f16, tag="gw")
                        nc.vector.tensor_tensor(
                            out=gw[:, 0 : se * OUT_F],
                            in0=t01[:, 0 : se * OUT_F],
                            in1=t23[:, 0 : se * OUT_F],
                            op=mybir.AluOpType.add,
                        )
                        # tree-reduce over k: 16 -> 8 -> 4 -> 2 -> 1
                        r1 = bp.tile([P, SUPER * 8 * OUT_F], f16, tag="r1")
                        _tree_add(nc, bass, mybir, r1[:, 0 : nb * 8 * OUT_F], gw[:], nb, 8, OUT_F)
                        r2 = bp.tile([P, SUPER * 4 * OUT_F], f16, tag="r2")
                        _tree_add(nc, bass, mybir, r2[:, 0 : nb * 4 * OUT_F], r1[:], nb, 4, OUT_F)
                        r3 = bp.tile([P, SUPER * 2 * OUT_F], f16, tag="r3")
                        _tree_add(nc, bass, mybir, r3[:, 0 : nb * 2 * OUT_F], r2[:], nb, 2, OUT_F)
                        r4 = bp.tile([P, SUPER, OUT_F], f32, tag="r4")
                        _tree_add(
                            nc, bass, mybir,
                            r4[:, 0:nb, :].rearrange("p a b -> p (a b)"),
                            r3[:], nb, 1, OUT_F,
                        )

                        out_rows = min(nb * P, NLOC - b0 * P)
                        full_s = out_rows // P
                        if full_s:
                            nc.sync.dma_start(
                                out=bass.AP(
                                    out_d[:].tensor,
                                    out_d[:].offset + b0 * P * OUT_F,
                                    [[OUT_F, P], [P * OUT_F, full_s], [1, OUT_F]],
                                ),
                                in_=r4[:, 0:full_s, :],
                            )
                        rem = out_rows - full_s * P
                        if rem:
                            nc.sync.dma_start(
                                out=bass.AP(
                                    out_d[:].tensor,
                                    out_d[:].offset + (b0 + full_s) * P * OUT_F,
                                    [[OUT_F, rem], [1, OUT_F]],
                                ),
                                in_=r4[0:rem, full_s, :],
                            )
                        b0 += nb

    nc.compile()
    return nc


def _tree_add(nc, bass, mybir, out, in_ap, s, half, outf):
    """out[p, s, j, f] = in[p, s, j, f] + in[p, s, j+half, f] for j in [0, half)."""
    lo = bass.AP(
        in_ap.tensor,
        in_ap.offset,
        [list(in_ap.ap[0]), [2 * half * outf, s], [outf, half], [1, outf]],
    )
    hi = bass.AP(
        in_ap.tensor,
        in_ap.offset + half * outf,
        [list(in_ap.ap[0]), [2 * half * outf, s], [outf, half], [1, outf]],
    )
    nc.vector.tensor_tensor(out=out, in0=lo, in1=hi, op=mybir.AluOpType.add)


# ---------------------------------------------------------------- host side
def _host_prep(input_h, W, a, bias, indices):
    """Build the 8 per-core in_maps. Index-side layout prep only."""
    idx = np.ascontiguousarray(indices.astype(np.int32))
    a2 = np.concatenate([a[OUT_F:], a[:OUT_F]], axis=1).astype(np.float32)  # [64,2]

    in_maps = []
    for c in range(NCORES):
        r0 = c * NLOC
        inT = np.ascontiguousarray(input_h[r0 : r0 + NLOC].T)
        ecols = idx[r0 * DEG : (r0 + NLOC) * DEG].reshape(NLOC, DEG)
        ep = np.zeros((NBLK * P, DEG), dtype=np.int64)
        ep[:NLOC] = ecols
        epb = ep.reshape(NBLK, P, DEG)                    # [blk, d, k]
        idx_cols = []
        oneh_parts = []
        b0 = 0
        for nb in SUPERS:
            cols = epb[b0 : b0 + nb]                      # [nb, d, k]
            cols = cols.transpose(0, 2, 1).reshape(nb * DEG * P)  # j = (b*16+k)*128+d
            colq = (cols >> 2).astype(np.int16)
            colm = (cols & 3)
            se = nb * DEG
            # wrapped-16 int16 layout, replicated across the 8 GpSimd cores
            w16 = colq.reshape(se * P // 16, 16).T        # [16, iw]
            idx_cols.append(np.tile(w16, (8, 1)))         # [128, iw]
            # one-hot of col&3 at the dest-aligned position: [d, slot, m]
            pm = colm.reshape(se, P)                      # [slot, d]
            oh = np.zeros((P, se, PACK), dtype=np.float16)
            qq, dd = np.meshgrid(np.arange(se), np.arange(P), indexing="ij")
            oh[dd, qq, pm[qq, dd]] = 1.0
            oneh_parts.append(oh.reshape(P, se * PACK))
            b0 += nb
        idx16 = np.concatenate(idx_cols, axis=1)          # [128, IWTOT]
        oneh = np.concatenate(oneh_parts, axis=1)         # [128, NBLK*DEG*PACK]

        in_maps.append(
            {
                "inT": inT.astype(np.float32),
                "W_in": np.asarray(W, dtype=np.float32),
                "a2_in": a2,
                "bias_in": np.asarray(bias, dtype=np.float32),
                "idx16w": np.ascontiguousarray(idx16),
                "oneh_in": np.ascontiguousarray(oneh),
            }
        )
    return in_maps


def _reference_numpy(input_h, W, a, bias, indptr, indices):
    """Exact CPU fallback mirroring the jax reference (used only if the CSR is
    not the uniform-degree layout this kernel is specialized for)."""
    h = input_h.astype(np.float64) @ W.astype(np.float64) + bias.astype(np.float64)
    deg = np.diff(indptr.astype(np.int64))
    row = np.repeat(np.arange(N, dtype=np.int64), deg)
    e_cnt = indices.shape[0]
    if row.shape[0] < e_cnt:
        pad_val = row[-1] if row.shape[0] else 0
        row = np.pad(row, (0, e_cnt - row.shape[0]), constant_values=pad_val)
    row = row[:e_cnt]
    col = indices.astype(np.int64)
    a_src = a[:OUT_F, 0].astype(np.float64)
    a_dst = a[OUT_F:, 0].astype(np.float64)
    wh1 = h @ a_src
    wh2 = h @ a_dst
    e = wh1[row] + wh2[col]
    e = np.where(e >= 0, e, ALPHA * e)
    emax = np.full(N, -np.inf)
    np.maximum.at(emax, row, e)
    ex = np.exp(e - emax[row])
    den = np.zeros(N)
    np.add.at(den, row, ex)
    attn = ex / (den[row] + EPS)
    out = np.zeros((N, OUT_F))
    np.add.at(out, row, attn[:, None] * h[col])
    return out.astype(np.float32)


def kernel(input_h, W, a, bias, indptr, indices):
    input_h = np.asarray(input_h, dtype=np.float32)
    W = np.asarray(W, dtype=np.float32)
    a = np.asarray(a, dtype=np.float32)
    bias = np.asarray(bias, dtype=np.float32)
    indptr = np.asarray(indptr)
    indices_np = np.asarray(indices)

    expected_indptr = np.arange(N + 1, dtype=np.int64) * DEG
    if (
        indptr.shape[0] != N + 1
        or indices_np.shape[0] != E
        or not np.array_equal(indptr.astype(np.int64), expected_indptr)
    ):
        return _reference_numpy(input_h, W, a, bias, indptr, indices_np)

    _install_ntff_shim()
    _install_dma_gather_patch()
    from concourse.bass_utils import run_bass_kernel_spmd

    key = "gat"
    if key not in _PROGRAM_CACHE:
        _PROGRAM_CACHE[key] = build_program()
    nc = _PROGRAM_CACHE[key]

    in_maps = _host_prep(input_h, W, a, bias, indices_np)
    res = run_bass_kernel_spmd(nc, in_maps, core_ids=list(range(NCORES)))
    out = np.concatenate([res.results[c]["out_d"] for c in range(NCORES)], axis=0)
    return out.astype(np.float32)


if __name__ == "__main__":
    pass


# revision 11
# speedup vs baseline: 1.0496x; 1.0496x over previous
"""GAT layer kernel for Trainium2 (8 NeuronCores, Bass/Tile).

Problem: h = input_h @ W + bias; per-edge e = leakyrelu(wh1[row] + wh2[col]);
segment softmax over each destination row's 16 edges; out = segment_sum of
attn * h[col].

Strategy: destination rows sharded across 8 cores (12500 each). Each core
computes h/wh1/wh2 for its own rows into a packed fp16 table (4 nodes per
768B row: per-node 96 fp16 = 64 feats + wh2 f32 bitcast), all-gathers it,
then fetches h[col] rows per edge with the GpSimd `dma_gather` ucode
(int16 idx = col>>2, 708B fetch covering all 4 node slots).

Layout: DEST-ALIGNED — idx position j = slot*128 + dest, so dest d's 16
edges land along the free dim of partition d (slot = block_local*16 + k).
Softmax is then pure per-partition work (Act exp + free-axis accumulator
for denominators, wh1 read straight from an SBUF-resident [128, NBLK]
tile), and aggregation+slot-selection run as 4 per-slot DVE products
(one-hot masks folded into fp16 attention weights) + halving tree-reduce.
Blocks are processed in supers of 7 (98 = 14*7) to amortize op overheads.

Host-side prep is limited to slicing/layout of the *index* input (wrapped
int16 col>>2 lists, col&3 one-hot masks). All numerical work on h/W/a/bias
happens on device.
"""

import sys
import types

import numpy as np

sys.path.insert(0, "/opt/trn_rl_repo")

# ---------------------------------------------------------------- constants
N = 100000
DEG = 16
E = N * DEG
IN_F = 128
OUT_F = 64
ALPHA = 0.2
EPS = 1e-12

NCORES = 8
NLOC = N // NCORES              # 12500 dest rows per core
P = 128
NBLK = (NLOC + P - 1) // P      # 98 blocks of 128 dests
LAST_VALID = NLOC - (NBLK - 1) * P  # 84 dests in last block

PACK = 4                        # nodes per table row
NPITCH = 96                     # fp16 elems per node slot (192B)
RSTRIDE = PACK * NPITCH         # 384 fp16 = 768B row stride
FETCH = (PACK - 1) * NPITCH + 66  # 354 fp16 = 708B fetched per edge
TROWS = N // PACK               # 25000 table rows
TROWS_LOC = NLOC // PACK        # 3125 local table rows

SUPER = 4                       # blocks per phase-B iteration
SUPERS = [4] * 24 + [2]         # 24*4 + 2 = 98 blocks
SE = SUPER * DEG                # 64 slots per partition per (full) super
IWTOT = NBLK * DEG * P // 16    # 12544 int16 idx columns total


def _install_ntff_shim():
    if "antenv.axon_hooks" in sys.modules:
        return
    try:
        from trn_agent_boot.trn_boot import _ntff_profile_via_ctypes

        hook = _ntff_profile_via_ctypes("/opt/axon/libaxon_pjrt.so")
    except Exception:
        hook = None
    mod = types.ModuleType("antenv.axon_hooks")
    mod.get_axon_ntff_profile_hook = lambda: hook
    mod.set_axon_ntff_profile_hook = lambda h: None
    sys.modules["antenv.axon_hooks"] = mod


def _install_dma_gather_patch():
    """Relax bass's elem_size%256 assert (ucode needs it only for transpose)."""
    import inspect
    import textwrap

    import concourse.bass as bass

    if getattr(bass.BassGpSimd.dma_gather, "_gat_patched", False):
        return
    src = textwrap.dedent(inspect.getsource(bass.BassGpSimd.dma_gather))
    old = """    assert (
        elem_size_bytes > 0 and elem_size_bytes % 256 == 0
    )  # transpose restriction"""
    new = """    assert elem_size_bytes > 0
    if transpose:
        assert elem_size_bytes % 256 == 0"""
    assert old in src, "dma_gather source changed; patch needs updating"
    src = src.replace(old, new)
    g = dict(bass.__dict__)
    exec(src, g)
    g["dma_gather"]._gat_patched = True
    bass.BassGpSimd.dma_gather = g["dma_gather"]


# ---------------------------------------------------------------- program
_PROGRAM_CACHE = {}


def build_program():
    _install_ntff_shim()
    _install_dma_gather_patch()
    import concourse.bacc as bacc
    import concourse.bass as bass
    import concourse.tile as tile
    from concourse import mybir

    f32 = mybir.dt.float32
    f16 = mybir.dt.float16
    i16 = mybir.dt.int16

    nc = bacc.Bacc(
        "TRN2",
        num_devices=NCORES,
        num_swdge_queues=4,
        detect_race_conditions=False,
    )

    # per-core external inputs
    inT = nc.dram_tensor("inT", [IN_F, NLOC], f32, kind="ExternalInput")
    W_in = nc.dram_tensor("W_in", [IN_F, OUT_F], f32, kind="ExternalInput")
    a2_in = nc.dram_tensor("a2_in", [OUT_F, 2], f32, kind="ExternalInput")
    bias_in = nc.dram_tensor("bias_in", [OUT_F], f32, kind="ExternalInput")
    idx16w = nc.dram_tensor("idx16w", [P, IWTOT], i16, kind="ExternalInput")
    oneh_in = nc.dram_tensor("oneh_in", [P, NBLK * DEG * PACK], f16, kind="ExternalInput")

    out_d = nc.dram_tensor("out_d", [NLOC, OUT_F], f32, kind="ExternalOutput")

    with tile.TileContext(nc) as tc:
        with tc.tile_pool(name="dram", bufs=1, space="DRAM") as dpool:
            h4_loc = dpool.tile([TROWS_LOC, RSTRIDE], f16)
            h4 = dpool.tile([TROWS, RSTRIDE], f16, addr_space="Shared")

            with tc.tile_pool(name="const", bufs=1) as cpool:
                # persistent: wrapped idx + one-hot masks + wh1
                idx_sb = cpool.tile([P, IWTOT], i16)
                nc.sync.dma_start(out=idx_sb[:], in_=idx16w[:])
                oneh_sb = cpool.tile([P, NBLK * DEG * PACK], f16)
                nc.sync.dma_start(out=oneh_sb[:], in_=oneh_in[:])
                wh1_all = cpool.tile([P, NBLK], f32)

                w_sb = cpool.tile([IN_F, OUT_F], f32)
                nc.sync.dma_start(out=w_sb[:], in_=W_in[:])
                a2_sb = cpool.tile([OUT_F, 2], f32)
                nc.sync.dma_start(out=a2_sb[:], in_=a2_in[:])
                bias_col = cpool.tile([OUT_F, 1], f32)
                nc.sync.dma_start(out=bias_col[:], in_=bias_in[:, None])
                bias_rep = cpool.tile([P, OUT_F], f32)
                nc.sync.dma_start(
                    out=bias_rep[:],
                    in_=bass.AP(
                        bias_in.handle if hasattr(bias_in, "handle") else bias_in,
                        0,
                        [[0, P], [1, OUT_F]],
                    ),
                )

                # Wa2 = W @ [a_dst | a_src] (contract over OUT_F): need W^T.
                with tc.tile_pool(name="pa", bufs=1, space="PSUM") as pp0, tc.tile_pool(
                    name="sa", bufs=1
                ) as sp0:
                    idp = sp0.tile([P, P], f32)
                    from concourse.masks import make_identity

                    make_identity(nc, idp[:])
                    wt_ps = pp0.tile([P, P], f32, space="PSUM")
                    nc.tensor.transpose(out=wt_ps[:OUT_F, :IN_F], in_=w_sb[:], identity=idp[:])
                    wt_sb = sp0.tile([OUT_F, IN_F], f32)
                    nc.vector.tensor_copy(out=wt_sb[:], in_=wt_ps[:OUT_F, :IN_F])
                    wa2_ps = pp0.tile([IN_F, 2], f32, space="PSUM")
                    nc.tensor.matmul(out=wa2_ps[:], lhsT=wt_sb[:], rhs=a2_sb[:])
                    wa2_sb = cpool.tile([IN_F, 2], f32)
                    nc.vector.tensor_copy(out=wa2_sb[:], in_=wa2_ps[:])
                    w66 = cpool.tile([IN_F, OUT_F + 2], f32)
                    nc.vector.tensor_copy(out=w66[:, 0:OUT_F], in_=w_sb[:])
                    nc.vector.tensor_copy(out=w66[:, OUT_F : OUT_F + 2], in_=wa2_ps[:])
                    # ab = a2^T bias -> [2,1]; broadcast each to 128 partitions
                    ab_ps = pp0.tile([2, 1], f32, space="PSUM")
                    nc.tensor.matmul(out=ab_ps[:], lhsT=a2_sb[:], rhs=bias_col[:])
                    ab_sb = sp0.tile([2, 1], f32)
                    nc.vector.tensor_copy(out=ab_sb[:], in_=ab_ps[:])
                    ab_dram = dpool.tile([2], f32)
                    nc.sync.dma_start(
                        out=bass.AP(ab_dram[:].tensor, ab_dram[:].offset, [[1, 2], [1, 1]]),
                        in_=ab_sb[:],
                    )
                    cv2_rep = cpool.tile([P, 1], f32)
                    cv1_rep = cpool.tile([P, 1], f32)
                    nc.sync.dma_start(
                        out=cv2_rep[:],
                        in_=bass.AP(ab_dram[:].tensor, ab_dram[:].offset, [[0, P], [1, 1]]),
                    )
                    nc.sync.dma_start(
                        out=cv1_rep[:],
                        in_=bass.AP(ab_dram[:].tensor, ab_dram[:].offset + 1, [[0, P], [1, 1]]),
                    )

                # ---------------- phase A: h / wh1 / wh2 for own rows
                with tc.tile_pool(name="pha_s", bufs=1) as spA, tc.tile_pool(
                    name="pha_ps", bufs=2, space="PSUM"
                ) as ppA, tc.tile_pool(name="pha_w", bufs=2) as wpA:
                    inT_sb = spA.tile([IN_F, NLOC], f32)
                    nc.sync.dma_start(out=inT_sb[:], in_=inT[:])

                    for t in range(NBLK):
                        r0 = t * P
                        rows = min(P, NLOC - r0)
                        lhsT = inT_sb[:, r0 : r0 + rows]
                        h_ps = ppA.tile([P, OUT_F + 2], f32, tag="h_ps")
                        nc.tensor.matmul(out=h_ps[:rows, :], lhsT=lhsT, rhs=w66[:])

                        h16 = wpA.tile([P, NPITCH], f16, tag="h16")
                        nc.vector.tensor_add(
                            out=h16[:rows, 0:OUT_F],
                            in0=h_ps[:rows, 0:OUT_F],
                            in1=bias_rep[:rows, :],
                        )
                        h16f32 = h16[:].bitcast(f32)
                        nc.vector.tensor_add(
                            out=h16f32[:rows, 32:33],
                            in0=h_ps[:rows, OUT_F : OUT_F + 1],
                            in1=cv2_rep[:rows, :],
                        )
                        nc.vector.tensor_add(
                            out=wh1_all[:rows, t : t + 1],
                            in0=h_ps[:rows, OUT_F + 1 : OUT_F + 2],
                            in1=cv1_rep[:rows, :],
                        )
                        # store 128 node rows = 32 table rows at node pitch
                        nc.sync.dma_start(
                            out=bass.AP(
                                h4_loc[:].tensor,
                                h4_loc[:].offset + (r0 // PACK) * RSTRIDE,
                                [[NPITCH, rows], [1, NPITCH]],
                            ),
                            in_=h16[:rows, :],
                        )

                # ---------------- all-gather the packed table
                nc.gpsimd.collective_compute(
                    "AllGather",
                    mybir.AluOpType.bypass,
                    replica_groups=[list(range(NCORES))],
                    ins=[h4_loc.opt()],
                    outs=[h4.opt()],
                )

                # ---------------- phase B
                with tc.tile_pool(name="phb_g", bufs=2) as gp, tc.tile_pool(
                    name="phb_s", bufs=2
                ) as bp, tc.tile_pool(name="phb_m", bufs=1) as mp:
                    b0 = 0
                    for s, nb in enumerate(SUPERS):
                        se = nb * DEG              # slots this super
                        nidx = se * P
                        iw0 = b0 * DEG * P // 16   # idx column offset
                        ihw = nidx // 16
                        G = gp.tile([P, SE, FETCH], f16, tag="G")
                        ncall = nidx // 1024       # 1024-idx calls (8 slots each)
                        for half in range(ncall):
                            nc.gpsimd.dma_gather(
                                out_ap=G[:, half * 8 : (half + 1) * 8, :],
                                in_ap=h4[:, 0:FETCH],
                                idxs_ap=idx_sb[
                                    :, iw0 + half * 64 : iw0 + (half + 1) * 64
                                ],
                                num_idxs=1024,
                                num_idxs_reg=1024,
                                elem_size=FETCH,
                                elem_step=RSTRIDE,
                                queue_num=(s * ncall + half) % 4,
                            )
                        oneh = bass.AP(
                            oneh_sb[:].tensor,
                            oneh_sb[:].offset + b0 * DEG * PACK,
                            [list(oneh_sb[:].ap[0]), [PACK, se], [1, PACK]],
                        )

                        Gf32 = G[:].bitcast(f32)  # [P, SE, FETCH//2]
                        # wh2 candidates (4 slots) -> compact on Act, select via oneh
                        wh2c = bp.tile([P, SE, PACK], f32, tag="wh2c")
                        nc.scalar.copy(
                            out=wh2c[:, 0:se, :].rearrange("p a b -> p (a b)"),
                            in_=bass.AP(
                                Gf32.tensor,
                                Gf32.offset + 32,
                                [list(Gf32.ap[0]), [FETCH // 2, se], [NPITCH // 2, PACK]],
                            ),
                        )
                        oneh32 = bp.tile([P, SE, PACK], f32, tag="oneh32")
                        nc.scalar.copy(
                            out=oneh32[:, 0:se, :].rearrange("p a b -> p (a b)"),
                            in_=oneh,
                        )
                        wh2s = bp.tile([P, SE, PACK], f32, tag="wh2s")
                        nc.vector.tensor_tensor(
                            out=wh2s[:, 0:se, :],
                            in0=wh2c[:, 0:se, :],
                            in1=oneh32[:, 0:se, :],
                            op=mybir.AluOpType.mult,
                        )
                        wh2e = bp.tile([P, SUPER, DEG], f32, tag="wh2e")
                        nc.vector.reduce_sum(
                            out=wh2e[:, 0:nb, :].rearrange("p a b -> p (a b)"),
                            in_=wh2s[:, 0:se, :],
                            axis=mybir.AxisListType.X,
                        )
                        # e0 = wh2e + wh1[dest] (wh1 broadcast along k)
                        e0 = bp.tile([P, SUPER, DEG], f32, tag="e0")
                        nc.vector.tensor_tensor(
                            out=e0[:, 0:nb, :],
                            in0=wh2e[:, 0:nb, :],
                            in1=bass.AP(
                                wh1_all[:].tensor,
                                wh1_all[:].offset + b0,
                                [list(wh1_all[:].ap[0]), [1, nb], [0, DEG]],
                            ),
                            op=mybir.AluOpType.add,
                        )
                        # leaky relu
                        esc = bp.tile([P, SE], f32, tag="esc")
                        nc.vector.tensor_scalar_mul(
                            out=esc[:, 0:se],
                            in0=e0[:, 0:nb, :].rearrange("p s k -> p (s k)"),
                            scalar1=ALPHA,
                        )
                        elr = bp.tile([P, SE], f32, tag="elr")
                        nc.vector.tensor_tensor(
                            out=elr[:, 0:se],
                            in0=e0[:, 0:nb, :].rearrange("p s k -> p (s k)"),
                            in1=esc[:, 0:se],
                            op=mybir.AluOpType.max,
                        )
                        # ex = exp(e); per-block denominators
                        ex = bp.tile([P, SUPER, DEG], f32, tag="ex")
                        nc.scalar.activation(
                            out=ex[:, 0:nb, :].rearrange("p s k -> p (s k)"),
                            in_=elr[:, 0:se],
                            func=mybir.ActivationFunctionType.Exp,
                        )
                        den = bp.tile([P, SUPER], f32, tag="den")
                        nc.vector.reduce_sum(
                            out=den[:, 0:nb], in_=ex[:, 0:nb, :], axis=mybir.AxisListType.X
                        )
                        dene = bp.tile([P, SUPER], f32, tag="dene")
                        nc.vector.tensor_scalar_add(
                            out=dene[:, 0:nb], in0=den[:, 0:nb], scalar1=EPS
                        )
                        rden = bp.tile([P, SUPER], f32, tag="rden")
                        nc.vector.reciprocal(out=rden[:, 0:nb], in_=dene[:, 0:nb])
                        # attn = ex * rden (fp16)
                        attn = bp.tile([P, SE], f16, tag="attn")
                        nc.vector.tensor_tensor(
                            out=attn[:, 0:se],
                            in0=ex[:, 0:nb, :].rearrange("p s k -> p (s k)"),
                            in1=bass.AP(
                                rden[:].tensor,
                                rden[:].offset,
                                [list(rden[:].ap[0]), [1, nb], [0, DEG]],
                            ),
                            op=mybir.AluOpType.mult,
                        )
                        # w[p, slot, m] = attn * oneh ; w8 = w expanded x8 (Act)
                        w = bp.tile([P, SE, PACK], f16, tag="w")
                        nc.vector.tensor_tensor(
                            out=w[:, 0:se, :],
                            in0=oneh,
                            in1=bass.AP(
                                attn[:].tensor,
                                attn[:].offset,
                                [list(attn[:].ap[0]), [1, se], [0, PACK]],
                            ),
                            op=mybir.AluOpType.mult,
                        )
                        w8 = bp.tile([P, SE, PACK, 8], f16, tag="w8")
                        nc.scalar.copy(
                            out=w8[:, 0:se, :, :].rearrange("p a b c -> p (a b c)"),
                            in_=bass.AP(
                                w[:].tensor,
                                w[:].offset,
                                [list(w[:].ap[0]), [1, se * PACK], [0, 8]],
                            ),
                        )
                        # per-slot products + m-tree
                        pma = mp.tile([P, SE, OUT_F], f16, tag="pma")
                        pmb = mp.tile([P, SE, OUT_F], f16, tag="pmb")
                        t01 = mp.tile([P, SE * OUT_F], f16, tag="t01")
                        t23 = mp.tile([P, SE * OUT_F], f16, tag="t23")
                        for mpair, (tdst, tsrc_a, tsrc_b) in enumerate(
                            [(t01, pma, pmb), (t23, pma, pmb)]
                        ):
                            for mm, pm in ((2 * mpair, tsrc_a), (2 * mpair + 1, tsrc_b)):
                                nc.vector.tensor_tensor(
                                    out=pm[:, 0:se, :].rearrange("p a b -> p (a b)"),
                                    in0=bass.AP(
                                        G[:].tensor,
                                        G[:].offset + mm * NPITCH,
                                        [list(G[:].ap[0]), [FETCH, se], [1, OUT_F]],
                                    ),
                                    in1=bass.AP(
                                        w8[:].tensor,
                                        w8[:].offset + mm * 8,
                                        [list(w8[:].ap[0]), [PACK * 8, se], [0, 8], [1, 8]],
                                    ),
                                    op=mybir.AluOpType.mult,
                                )
                            nc.vector.tensor_tensor(
                                out=tdst[:, 0 : se * OUT_F],
                                in0=tsrc_a[:, 0:se, :].rearrange("p a b -> p (a b)"),
                                in1=tsrc_b[:, 0:se, :].rearrange("p a b -> p (a b)"),
                                op=mybir.AluOpType.add,
                            )
                        gw = mp.tile([P, SE * OUT_F], f16, tag="gw")
                        nc.vector.tensor_tensor(
                            out=gw[:, 0 : se * OUT_F],
                            in0=t01[:, 0 : se * OUT_F],
                            in1=t23[:, 0 : se * OUT_F],
                            op=mybir.AluOpType.add,
                        )
                        # tree-reduce over k: 16 -> 8 -> 4 -> 2 -> 1
                        r1 = bp.tile([P, SUPER * 8 * OUT_F], f16, tag="r1")
                        _tree_add(nc, bass, mybir, r1[:, 0 : nb * 8 * OUT_F], gw[:], nb, 8, OUT_F)
                        r2 = bp.tile([P, SUPER * 4 * OUT_F], f16, tag="r2")
                        _tree_add(nc, bass, mybir, r2[:, 0 : nb * 4 * OUT_F], r1[:], nb, 4, OUT_F)
                        r3 = bp.tile([P, SUPER * 2 * OUT_F], f16, tag="r3")
                        _tree_add(nc, bass, mybir, r3[:, 0 : nb * 2 * OUT_F], r2[:], nb, 2, OUT_F)
                        r4 = bp.tile([P, SUPER, OUT_F], f32, tag="r4")
                        _tree_add(
                            nc, bass, mybir,
                            r4[:, 0:nb, :].rearrange("p a b -> p (a b)"),
                            r3[:], nb, 1, OUT_F,
                        )

                        out_rows = min(nb * P, NLOC - b0 * P)
                        full_s = out_rows // P
                        if full_s:
                            nc.sync.dma_start(
                                out=bass.AP(
                                    out_d[:].tensor,
                                    out_d[:].offset + b0 * P * OUT_F,
                                    [[OUT_F, P], [P * OUT_F, full_s], [1, OUT_F]],
                                ),
                                in_=r4[:, 0:full_s, :],
                            )
                        rem = out_rows - full_s * P
                        if rem:
                            nc.sync.dma_start(
                                out=bass.AP(
                                    out_d[:].tensor,
                                    out_d[:].offset + (b0 + full_s) * P * OUT_F,
                                    [[OUT_F, rem], [1, OUT_F]],
                                ),
                                in_=r4[0:rem, full_s, :],
                            )
                        b0 += nb

    nc.compile()
    return nc


def _tree_add(nc, bass, mybir, out, in_ap, s, half, outf):
    """out[p, s, j, f] = in[p, s, j, f] + in[p, s, j+half, f] for j in [0, half)."""
    lo = bass.AP(
        in_ap.tensor,
        in_ap.offset,
        [list(in_ap.ap[0]), [2 * half * outf, s], [outf, half], [1, outf]],
    )
    hi = bass.AP(
        in_ap.tensor,
        in_ap.offset + half * outf,
        [list(in_ap.ap[0]), [2 * half * outf, s], [outf, half], [1, outf]],
    )
    nc.vector.tensor_tensor(out=out, in0=lo, in1=hi, op=mybir.AluOpType.add)


# ---------------------------------------------------------------- host side
def _host_prep(input_h, W, a, bias, indices):
    """Build the 8 per-core in_maps. Index-side layout prep only."""
    idx = np.ascontiguousarray(indices.astype(np.int32))
    a2 = np.concatenate([a[OUT_F:], a[:OUT_F]], axis=1).astype(np.float32)  # [64,2]

    in_maps = []
    for c in range(NCORES):
        r0 = c * NLOC
        inT = np.ascontiguousarray(input_h[r0 : r0 + NLOC].T)
        ecols = idx[r0 * DEG : (r0 + NLOC) * DEG].reshape(NLOC, DEG)
        ep = np.zeros((NBLK * P, DEG), dtype=np.int64)
        ep[:NLOC] = ecols
        epb = ep.reshape(NBLK, P, DEG)                    # [blk, d, k]
        idx_cols = []
        oneh_parts = []
        b0 = 0
        for nb in SUPERS:
            cols = epb[b0 : b0 + nb]                      # [nb, d, k]
            cols = cols.transpose(0, 2, 1).reshape(nb * DEG * P)  # j = (b*16+k)*128+d
            colq = (cols >> 2).astype(np.int16)
            colm = (cols & 3)
            se = nb * DEG
            # wrapped-16 int16 layout, replicated across the 8 GpSimd cores
            w16 = colq.reshape(se * P // 16, 16).T        # [16, iw]
            idx_cols.append(np.tile(w16, (8, 1)))         # [128, iw]
            # one-hot of col&3 at the dest-aligned position: [d, slot, m]
            pm = colm.reshape(se, P)                      # [slot, d]
            oh = np.zeros((P, se, PACK), dtype=np.float16)
            qq, dd = np.meshgrid(np.arange(se), np.arange(P), indexing="ij")
            oh[dd, qq, pm[qq, dd]] = 1.0
            oneh_parts.append(oh.reshape(P, se * PACK))
            b0 += nb
        idx16 = np.concatenate(idx_cols, axis=1)          # [128, IWTOT]
        oneh = np.concatenate(oneh_parts, axis=1)         # [128, NBLK*DEG*PACK]

        in_maps.append(
            {
                "inT": inT.astype(np.float32),
                "W_in": np.asarray(W, dtype=np.float32),
                "a2_in": a2,
                "bias_in": np.asarray(bias, dtype=np.float32),
                "idx16w": np.ascontiguousarray(idx16),
                "oneh_in": np.ascontiguousarray(oneh),
            }
        )
    return in_maps


def _reference_numpy(input_h, W, a, bias, indptr, indices):
    """Exact CPU fallback mirroring the jax reference (used only if the CSR is
    not the uniform-degree layout this kernel is specialized for)."""
    h = input_h.astype(np.float64) @ W.astype(np.float64) + bias.astype(np.float64)
    deg = np.diff(indptr.astype(np.int64))
    row = np.repeat(np.arange(N, dtype=np.int64), deg)
    e_cnt = indices.shape[0]
    if row.shape[0] < e_cnt:
        pad_val = row[-1] if row.shape[0] else 0
        row = np.pad(row, (0, e_cnt - row.shape[0]), constant_values=pad_val)
    row = row[:e_cnt]
    col = indices.astype(np.int64)
    a_src = a[:OUT_F, 0].astype(np.float64)
    a_dst = a[OUT_F:, 0].astype(np.float64)
    wh1 = h @ a_src
    wh2 = h @ a_dst
    e = wh1[row] + wh2[col]
    e = np.where(e >= 0, e, ALPHA * e)
    emax = np.full(N, -np.inf)
    np.maximum.at(emax, row, e)
    ex = np.exp(e - emax[row])
    den = np.zeros(N)
    np.add.at(den, row, ex)
    attn = ex / (den[row] + EPS)
    out = np.zeros((N, OUT_F))
    np.add.at(out, row, attn[:, None] * h[col])
    return out.astype(np.float32)


def kernel(input_h, W, a, bias, indptr, indices):
    input_h = np.asarray(input_h, dtype=np.float32)
    W = np.asarray(W, dtype=np.float32)
    a = np.asarray(a, dtype=np.float32)
    bias = np.asarray(bias, dtype=np.float32)
    indptr = np.asarray(indptr)
    indices_np = np.asarray(indices)

    expected_indptr = np.arange(N + 1, dtype=np.int64) * DEG
    if (
        indptr.shape[0] != N + 1
        or indices_np.shape[0] != E
        or not np.array_equal(indptr.astype(np.int64), expected_indptr)
    ):
        return _reference_numpy(input_h, W, a, bias, indptr, indices_np)

    _install_ntff_shim()
    _install_dma_gather_patch()
    from concourse.bass_utils import run_bass_kernel_spmd

    key = "gat"
    if key not in _PROGRAM_CACHE:
        _PROGRAM_CACHE[key] = build_program()
    nc = _PROGRAM_CACHE[key]

    in_maps = _host_prep(input_h, W, a, bias, indices_np)
    res = run_bass_kernel_spmd(nc, in_maps, core_ids=list(range(NCORES)))
    out = np.concatenate([res.results[c]["out_d"] for c in range(NCORES)], axis=0)
    return out.astype(np.float32)


if __name__ == "__main__":
    pass


# revision 12
# speedup vs baseline: 1.0744x; 1.0236x over previous
"""GAT layer kernel for Trainium2 (8 NeuronCores, Bass/Tile).

Problem: h = input_h @ W + bias; per-edge e = leakyrelu(wh1[row] + wh2[col]);
segment softmax over each destination row's 16 edges; out = segment_sum of
attn * h[col].

Strategy: destination rows sharded across 8 cores (12500 each). Each core
computes h/wh1/wh2 for its own rows into a packed fp16 table (4 nodes per
768B row: per-node 96 fp16 = 64 feats + wh2 f32 bitcast), all-gathers it,
then fetches h[col] rows per edge with the GpSimd `dma_gather` ucode
(int16 idx = col>>2, 708B fetch covering all 4 node slots).

Layout: DEST-ALIGNED — idx position j = slot*128 + dest, so dest d's 16
edges land along the free dim of partition d (slot = block_local*16 + k).
Softmax is then pure per-partition work (Act exp + free-axis accumulator
for denominators, wh1 read straight from an SBUF-resident [128, NBLK]
tile), and aggregation+slot-selection run as 4 per-slot DVE products
(one-hot masks folded into fp16 attention weights) + halving tree-reduce.
Blocks are processed in supers of 7 (98 = 14*7) to amortize op overheads.

Host-side prep is limited to slicing/layout of the *index* input (wrapped
int16 col>>2 lists, col&3 one-hot masks). All numerical work on h/W/a/bias
happens on device.
"""

import sys
import types

import numpy as np

sys.path.insert(0, "/opt/trn_rl_repo")

# ---------------------------------------------------------------- constants
N = 100000
DEG = 16
E = N * DEG
IN_F = 128
OUT_F = 64
ALPHA = 0.2
EPS = 1e-12

NCORES = 8
NLOC = N // NCORES              # 12500 dest rows per core
P = 128
NBLK = (NLOC + P - 1) // P      # 98 blocks of 128 dests
LAST_VALID = NLOC - (NBLK - 1) * P  # 84 dests in last block

PACK = 4                        # nodes per table row
NPITCH = 96                     # fp16 elems per node slot (192B)
RSTRIDE = PACK * NPITCH         # 384 fp16 = 768B row stride
FETCH = (PACK - 1) * NPITCH + 66  # 354 fp16 = 708B fetched per edge
TROWS = N // PACK               # 25000 table rows
TROWS_LOC = NLOC // PACK        # 3125 local table rows

SUPER = 3                       # blocks per phase-B iteration
SUPERS = [3] * 32 + [2]         # 32*3 + 2 = 98 blocks
SE = SUPER * DEG                # 64 slots per partition per (full) super
IWTOT = NBLK * DEG * P // 16    # 12544 int16 idx columns total


def _install_ntff_shim():
    if "antenv.axon_hooks" in sys.modules:
        return
    try:
        from trn_agent_boot.trn_boot import _ntff_profile_via_ctypes

        hook = _ntff_profile_via_ctypes("/opt/axon/libaxon_pjrt.so")
    except Exception:
        hook = None
    mod = types.ModuleType("antenv.axon_hooks")
    mod.get_axon_ntff_profile_hook = lambda: hook
    mod.set_axon_ntff_profile_hook = lambda h: None
    sys.modules["antenv.axon_hooks"] = mod


def _install_dma_gather_patch():
    """Relax bass's elem_size%256 assert (ucode needs it only for transpose)."""
    import inspect
    import textwrap

    import concourse.bass as bass

    if getattr(bass.BassGpSimd.dma_gather, "_gat_patched", False):
        return
    src = textwrap.dedent(inspect.getsource(bass.BassGpSimd.dma_gather))
    old = """    assert (
        elem_size_bytes > 0 and elem_size_bytes % 256 == 0
    )  # transpose restriction"""
    new = """    assert elem_size_bytes > 0
    if transpose:
        assert elem_size_bytes % 256 == 0"""
    assert old in src, "dma_gather source changed; patch needs updating"
    src = src.replace(old, new)
    g = dict(bass.__dict__)
    exec(src, g)
    g["dma_gather"]._gat_patched = True
    bass.BassGpSimd.dma_gather = g["dma_gather"]


# ---------------------------------------------------------------- program
_PROGRAM_CACHE = {}


def build_program():
    _install_ntff_shim()
    _install_dma_gather_patch()
    import concourse.bacc as bacc
    import concourse.bass as bass
    import concourse.tile as tile
    from concourse import mybir

    f32 = mybir.dt.float32
    f16 = mybir.dt.float16
    i16 = mybir.dt.int16

    nc = bacc.Bacc(
        "TRN2",
        num_devices=NCORES,
        num_swdge_queues=4,
        detect_race_conditions=False,
    )

    # per-core external inputs
    inT = nc.dram_tensor("inT", [IN_F, NLOC], f32, kind="ExternalInput")
    W_in = nc.dram_tensor("W_in", [IN_F, OUT_F], f32, kind="ExternalInput")
    a2_in = nc.dram_tensor("a2_in", [OUT_F, 2], f32, kind="ExternalInput")
    bias_in = nc.dram_tensor("bias_in", [OUT_F], f32, kind="ExternalInput")
    idx16w = nc.dram_tensor("idx16w", [P, IWTOT], i16, kind="ExternalInput")
    oneh_in = nc.dram_tensor("oneh_in", [P, NBLK * DEG * PACK], f16, kind="ExternalInput")

    out_d = nc.dram_tensor("out_d", [NLOC, OUT_F], f32, kind="ExternalOutput")

    with tile.TileContext(nc) as tc:
        with tc.tile_pool(name="dram", bufs=1, space="DRAM") as dpool:
            h4_loc = dpool.tile([TROWS_LOC, RSTRIDE], f16)
            h4 = dpool.tile([TROWS, RSTRIDE], f16, addr_space="Shared")

            with tc.tile_pool(name="const", bufs=1) as cpool:
                # persistent: wrapped idx + one-hot masks + wh1
                idx_sb = cpool.tile([P, IWTOT], i16)
                nc.sync.dma_start(out=idx_sb[:], in_=idx16w[:])
                oneh_sb = cpool.tile([P, NBLK * DEG * PACK], f16)
                nc.sync.dma_start(out=oneh_sb[:], in_=oneh_in[:])
                wh1_all = cpool.tile([P, NBLK], f32)

                w_sb = cpool.tile([IN_F, OUT_F], f32)
                nc.sync.dma_start(out=w_sb[:], in_=W_in[:])
                a2_sb = cpool.tile([OUT_F, 2], f32)
                nc.sync.dma_start(out=a2_sb[:], in_=a2_in[:])
                bias_col = cpool.tile([OUT_F, 1], f32)
                nc.sync.dma_start(out=bias_col[:], in_=bias_in[:, None])
                bias_rep = cpool.tile([P, OUT_F], f32)
                nc.sync.dma_start(
                    out=bias_rep[:],
                    in_=bass.AP(
                        bias_in.handle if hasattr(bias_in, "handle") else bias_in,
                        0,
                        [[0, P], [1, OUT_F]],
                    ),
                )

                # Wa2 = W @ [a_dst | a_src] (contract over OUT_F): need W^T.
                with tc.tile_pool(name="pa", bufs=1, space="PSUM") as pp0, tc.tile_pool(
                    name="sa", bufs=1
                ) as sp0:
                    idp = sp0.tile([P, P], f32)
                    from concourse.masks import make_identity

                    make_identity(nc, idp[:])
                    wt_ps = pp0.tile([P, P], f32, space="PSUM")
                    nc.tensor.transpose(out=wt_ps[:OUT_F, :IN_F], in_=w_sb[:], identity=idp[:])
                    wt_sb = sp0.tile([OUT_F, IN_F], f32)
                    nc.vector.tensor_copy(out=wt_sb[:], in_=wt_ps[:OUT_F, :IN_F])
                    wa2_ps = pp0.tile([IN_F, 2], f32, space="PSUM")
                    nc.tensor.matmul(out=wa2_ps[:], lhsT=wt_sb[:], rhs=a2_sb[:])
                    wa2_sb = cpool.tile([IN_F, 2], f32)
                    nc.vector.tensor_copy(out=wa2_sb[:], in_=wa2_ps[:])
                    w66 = cpool.tile([IN_F, OUT_F + 2], f32)
                    nc.vector.tensor_copy(out=w66[:, 0:OUT_F], in_=w_sb[:])
                    nc.vector.tensor_copy(out=w66[:, OUT_F : OUT_F + 2], in_=wa2_ps[:])
                    # ab = a2^T bias -> [2,1]; broadcast each to 128 partitions
                    ab_ps = pp0.tile([2, 1], f32, space="PSUM")
                    nc.tensor.matmul(out=ab_ps[:], lhsT=a2_sb[:], rhs=bias_col[:])
                    ab_sb = sp0.tile([2, 1], f32)
                    nc.vector.tensor_copy(out=ab_sb[:], in_=ab_ps[:])
                    ab_dram = dpool.tile([2], f32)
                    nc.sync.dma_start(
                        out=bass.AP(ab_dram[:].tensor, ab_dram[:].offset, [[1, 2], [1, 1]]),
                        in_=ab_sb[:],
                    )
                    cv2_rep = cpool.tile([P, 1], f32)
                    cv1_rep = cpool.tile([P, 1], f32)
                    nc.sync.dma_start(
                        out=cv2_rep[:],
                        in_=bass.AP(ab_dram[:].tensor, ab_dram[:].offset, [[0, P], [1, 1]]),
                    )
                    nc.sync.dma_start(
                        out=cv1_rep[:],
                        in_=bass.AP(ab_dram[:].tensor, ab_dram[:].offset + 1, [[0, P], [1, 1]]),
                    )

                # ---------------- phase A: h / wh1 / wh2 for own rows
                with tc.tile_pool(name="pha_s", bufs=1) as spA, tc.tile_pool(
                    name="pha_ps", bufs=2, space="PSUM"
                ) as ppA, tc.tile_pool(name="pha_w", bufs=2) as wpA:
                    inT_sb = spA.tile([IN_F, NLOC], f32)
                    nc.sync.dma_start(out=inT_sb[:], in_=inT[:])

                    for t in range(NBLK):
                        r0 = t * P
                        rows = min(P, NLOC - r0)
                        lhsT = inT_sb[:, r0 : r0 + rows]
                        h_ps = ppA.tile([P, OUT_F + 2], f32, tag="h_ps")
                        nc.tensor.matmul(out=h_ps[:rows, :], lhsT=lhsT, rhs=w66[:])

                        h16 = wpA.tile([P, NPITCH], f16, tag="h16")
                        nc.vector.tensor_add(
                            out=h16[:rows, 0:OUT_F],
                            in0=h_ps[:rows, 0:OUT_F],
                            in1=bias_rep[:rows, :],
                        )
                        h16f32 = h16[:].bitcast(f32)
                        nc.vector.tensor_add(
                            out=h16f32[:rows, 32:33],
                            in0=h_ps[:rows, OUT_F : OUT_F + 1],
                            in1=cv2_rep[:rows, :],
                        )
                        nc.vector.tensor_add(
                            out=wh1_all[:rows, t : t + 1],
                            in0=h_ps[:rows, OUT_F + 1 : OUT_F + 2],
                            in1=cv1_rep[:rows, :],
                        )
                        # store 128 node rows = 32 table rows at node pitch
                        nc.sync.dma_start(
                            out=bass.AP(
                                h4_loc[:].tensor,
                                h4_loc[:].offset + (r0 // PACK) * RSTRIDE,
                                [[NPITCH, rows], [1, NPITCH]],
                            ),
                            in_=h16[:rows, :],
                        )

                # ---------------- all-gather the packed table
                nc.gpsimd.collective_compute(
                    "AllGather",
                    mybir.AluOpType.bypass,
                    replica_groups=[list(range(NCORES))],
                    ins=[h4_loc.opt()],
                    outs=[h4.opt()],
                )

                # ---------------- phase B
                with tc.tile_pool(name="phb_g", bufs=3) as gp, tc.tile_pool(
                    name="phb_s", bufs=2
                ) as bp, tc.tile_pool(name="phb_m", bufs=1) as mp:
                    b0 = 0
                    for s, nb in enumerate(SUPERS):
                        se = nb * DEG              # slots this super
                        nidx = se * P
                        iw0 = b0 * DEG * P // 16   # idx column offset
                        ihw = nidx // 16
                        G = gp.tile([P, SE, FETCH], f16, tag="G")
                        ncall = nidx // 1024       # 1024-idx calls (8 slots each)
                        for half in range(ncall):
                            nc.gpsimd.dma_gather(
                                out_ap=G[:, half * 8 : (half + 1) * 8, :],
                                in_ap=h4[:, 0:FETCH],
                                idxs_ap=idx_sb[
                                    :, iw0 + half * 64 : iw0 + (half + 1) * 64
                                ],
                                num_idxs=1024,
                                num_idxs_reg=1024,
                                elem_size=FETCH,
                                elem_step=RSTRIDE,
                                queue_num=(s * ncall + half) % 4,
                            )
                        oneh = bass.AP(
                            oneh_sb[:].tensor,
                            oneh_sb[:].offset + b0 * DEG * PACK,
                            [list(oneh_sb[:].ap[0]), [PACK, se], [1, PACK]],
                        )

                        Gf32 = G[:].bitcast(f32)  # [P, SE, FETCH//2]
                        # wh2 candidates (4 slots) -> compact on Act, select via oneh
                        wh2c = bp.tile([P, SE, PACK], f32, tag="wh2c")
                        nc.scalar.copy(
                            out=wh2c[:, 0:se, :].rearrange("p a b -> p (a b)"),
                            in_=bass.AP(
                                Gf32.tensor,
                                Gf32.offset + 32,
                                [list(Gf32.ap[0]), [FETCH // 2, se], [NPITCH // 2, PACK]],
                            ),
                        )
                        oneh32 = bp.tile([P, SE, PACK], f32, tag="oneh32")
                        nc.scalar.copy(
                            out=oneh32[:, 0:se, :].rearrange("p a b -> p (a b)"),
                            in_=oneh,
                        )
                        wh2s = bp.tile([P, SE, PACK], f32, tag="wh2s")
                        nc.vector.tensor_tensor(
                            out=wh2s[:, 0:se, :],
                            in0=wh2c[:, 0:se, :],
                            in1=oneh32[:, 0:se, :],
                            op=mybir.AluOpType.mult,
                        )
                        wh2e = bp.tile([P, SUPER, DEG], f32, tag="wh2e")
                        nc.vector.reduce_sum(
                            out=wh2e[:, 0:nb, :].rearrange("p a b -> p (a b)"),
                            in_=wh2s[:, 0:se, :],
                            axis=mybir.AxisListType.X,
                        )
                        # e0 = wh2e + wh1[dest] (wh1 broadcast along k)
                        e0 = bp.tile([P, SUPER, DEG], f32, tag="e0")
                        nc.vector.tensor_tensor(
                            out=e0[:, 0:nb, :],
                            in0=wh2e[:, 0:nb, :],
                            in1=bass.AP(
                                wh1_all[:].tensor,
                                wh1_all[:].offset + b0,
                                [list(wh1_all[:].ap[0]), [1, nb], [0, DEG]],
                            ),
                            op=mybir.AluOpType.add,
                        )
                        # leaky relu
                        esc = bp.tile([P, SE], f32, tag="esc")
                        nc.vector.tensor_scalar_mul(
                            out=esc[:, 0:se],
                            in0=e0[:, 0:nb, :].rearrange("p s k -> p (s k)"),
                            scalar1=ALPHA,
                        )
                        elr = bp.tile([P, SE], f32, tag="elr")
                        nc.vector.tensor_tensor(
                            out=elr[:, 0:se],
                            in0=e0[:, 0:nb, :].rearrange("p s k -> p (s k)"),
                            in1=esc[:, 0:se],
                            op=mybir.AluOpType.max,
                        )
                        # ex = exp(e); per-block denominators
                        ex = bp.tile([P, SUPER, DEG], f32, tag="ex")
                        nc.scalar.activation(
                            out=ex[:, 0:nb, :].rearrange("p s k -> p (s k)"),
                            in_=elr[:, 0:se],
                            func=mybir.ActivationFunctionType.Exp,
                        )
                        den = bp.tile([P, SUPER], f32, tag="den")
                        nc.vector.reduce_sum(
                            out=den[:, 0:nb], in_=ex[:, 0:nb, :], axis=mybir.AxisListType.X
                        )
                        dene = bp.tile([P, SUPER], f32, tag="dene")
                        nc.vector.tensor_scalar_add(
                            out=dene[:, 0:nb], in0=den[:, 0:nb], scalar1=EPS
                        )
                        rden = bp.tile([P, SUPER], f32, tag="rden")
                        nc.vector.reciprocal(out=rden[:, 0:nb], in_=dene[:, 0:nb])
                        # attn = ex * rden (fp16)
                        attn = bp.tile([P, SE], f16, tag="attn")
                        nc.vector.tensor_tensor(
                            out=attn[:, 0:se],
                            in0=ex[:, 0:nb, :].rearrange("p s k -> p (s k)"),
                            in1=bass.AP(
                                rden[:].tensor,
                                rden[:].offset,
                                [list(rden[:].ap[0]), [1, nb], [0, DEG]],
                            ),
                            op=mybir.AluOpType.mult,
                        )
                        # w[p, slot, m] = attn * oneh ; w8 = w expanded x8 (Act)
                        w = bp.tile([P, SE, PACK], f16, tag="w")
                        nc.vector.tensor_tensor(
                            out=w[:, 0:se, :],
                            in0=oneh,
                            in1=bass.AP(
                                attn[:].tensor,
                                attn[:].offset,
                                [list(attn[:].ap[0]), [1, se], [0, PACK]],
                            ),
                            op=mybir.AluOpType.mult,
                        )
                        w8 = bp.tile([P, SE, PACK, 8], f16, tag="w8")
                        nc.scalar.copy(
                            out=w8[:, 0:se, :, :].rearrange("p a b c -> p (a b c)"),
                            in_=bass.AP(
                                w[:].tensor,
                                w[:].offset,
                                [list(w[:].ap[0]), [1, se * PACK], [0, 8]],
                            ),
                        )
                        # per-slot products + m-tree
                        pma = mp.tile([P, SE, OUT_F], f16, tag="pma")
                        pmb = mp.tile([P, SE, OUT_F], f16, tag="pmb")
                        t01 = mp.tile([P, SE * OUT_F], f16, tag="t01")
                        t23 = mp.tile([P, SE * OUT_F], f16, tag="t23")
                        for mpair, (tdst, tsrc_a, tsrc_b) in enumerate(
                            [(t01, pma, pmb), (t23, pma, pmb)]
                        ):
                            for mm, pm in ((2 * mpair, tsrc_a), (2 * mpair + 1, tsrc_b)):
                                nc.vector.tensor_tensor(
                                    out=pm[:, 0:se, :].rearrange("p a b -> p (a b)"),
                                    in0=bass.AP(
                                        G[:].tensor,
                                        G[:].offset + mm * NPITCH,
                                        [list(G[:].ap[0]), [FETCH, se], [1, OUT_F]],
                                    ),
                                    in1=bass.AP(
                                        w8[:].tensor,
                                        w8[:].offset + mm * 8,
                                        [list(w8[:].ap[0]), [PACK * 8, se], [0, 8], [1, 8]],
                                    ),
                                    op=mybir.AluOpType.mult,
                                )
                            nc.vector.tensor_tensor(
                                out=tdst[:, 0 : se * OUT_F],
                                in0=tsrc_a[:, 0:se, :].rearrange("p a b -> p (a b)"),
                                in1=tsrc_b[:, 0:se, :].rearrange("p a b -> p (a b)"),
                                op=mybir.AluOpType.add,
                            )
                        gw = mp.tile([P, SE * OUT_F], f16, tag="gw")
                        nc.vector.tensor_tensor(
                            out=gw[:, 0 : se * OUT_F],
                            in0=t01[:, 0 : se * OUT_F],
                            in1=t23[:, 0 : se * OUT_F],
                            op=mybir.AluOpType.add,
                        )
                        # tree-reduce over k: 16 -> 8 -> 4 -> 2 -> 1
                        r1 = bp.tile([P, SUPER * 8 * OUT_F], f16, tag="r1")
                        _tree_add(nc, bass, mybir, r1[:, 0 : nb * 8 * OUT_F], gw[:], nb, 8, OUT_F)
                        r2 = bp.tile([P, SUPER * 4 * OUT_F], f16, tag="r2")
                        _tree_add(nc, bass, mybir, r2[:, 0 : nb * 4 * OUT_F], r1[:], nb, 4, OUT_F)
                        r3 = bp.tile([P, SUPER * 2 * OUT_F], f16, tag="r3")
                        _tree_add(nc, bass, mybir, r3[:, 0 : nb * 2 * OUT_F], r2[:], nb, 2, OUT_F)
                        r4 = bp.tile([P, SUPER, OUT_F], f32, tag="r4")
                        _tree_add(
                            nc, bass, mybir,
                            r4[:, 0:nb, :].rearrange("p a b -> p (a b)"),
                            r3[:], nb, 1, OUT_F,
                        )

                        out_rows = min(nb * P, NLOC - b0 * P)
                        full_s = out_rows // P
                        if full_s:
                            nc.sync.dma_start(
                                out=bass.AP(
                                    out_d[:].tensor,
                                    out_d[:].offset + b0 * P * OUT_F,
                                    [[OUT_F, P], [P * OUT_F, full_s], [1, OUT_F]],
                                ),
                                in_=r4[:, 0:full_s, :],
                            )
                        rem = out_rows - full_s * P
                        if rem:
                            nc.sync.dma_start(
                                out=bass.AP(
                                    out_d[:].tensor,
                                    out_d[:].offset + (b0 + full_s) * P * OUT_F,
                                    [[OUT_F, rem], [1, OUT_F]],
                                ),
                                in_=r4[0:rem, full_s, :],
                            )
                        b0 += nb

    nc.compile()
    return nc


def _tree_add(nc, bass, mybir, out, in_ap, s, half, outf):
    """out[p, s, j, f] = in[p, s, j, f] + in[p, s, j+half, f] for j in [0, half)."""
    lo = bass.AP(
        in_ap.tensor,
        in_ap.offset,
        [list(in_ap.ap[0]), [2 * half * outf, s], [outf, half], [1, outf]],
    )
    hi = bass.AP(
        in_ap.tensor,
        in_ap.offset + half * outf,
        [list(in_ap.ap[0]), [2 * half * outf, s], [outf, half], [1, outf]],
    )
    nc.vector.tensor_tensor(out=out, in0=lo, in1=hi, op=mybir.AluOpType.add)


# ---------------------------------------------------------------- host side
def _host_prep(input_h, W, a, bias, indices):
    """Build the 8 per-core in_maps. Index-side layout prep only."""
    idx = np.ascontiguousarray(indices.astype(np.int32))
    a2 = np.concatenate([a[OUT_F:], a[:OUT_F]], axis=1).astype(np.float32)  # [64,2]

    in_maps = []
    for c in range(NCORES):
        r0 = c * NLOC
        inT = np.ascontiguousarray(input_h[r0 : r0 + NLOC].T)
        ecols = idx[r0 * DEG : (r0 + NLOC) * DEG].reshape(NLOC, DEG)
        ep = np.zeros((NBLK * P, DEG), dtype=np.int64)
        ep[:NLOC] = ecols
        epb = ep.reshape(NBLK, P, DEG)                    # [blk, d, k]
        idx_cols = []
        oneh_parts = []
        b0 = 0
        for nb in SUPERS:
            cols = epb[b0 : b0 + nb]                      # [nb, d, k]
            cols = cols.transpose(0, 2, 1).reshape(nb * DEG * P)  # j = (b*16+k)*128+d
            colq = (cols >> 2).astype(np.int16)
            colm = (cols & 3)
            se = nb * DEG
            # wrapped-16 int16 layout, replicated across the 8 GpSimd cores
            w16 = colq.reshape(se * P // 16, 16).T        # [16, iw]
            idx_cols.append(np.tile(w16, (8, 1)))         # [128, iw]
            # one-hot of col&3 at the dest-aligned position: [d, slot, m]
            pm = colm.reshape(se, P)                      # [slot, d]
            oh = np.zeros((P, se, PACK), dtype=np.float16)
            qq, dd = np.meshgrid(np.arange(se), np.arange(P), indexing="ij")
            oh[dd, qq, pm[qq, dd]] = 1.0
            oneh_parts.append(oh.reshape(P, se * PACK))
            b0 += nb
        idx16 = np.concatenate(idx_cols, axis=1)          # [128, IWTOT]
        oneh = np.concatenate(oneh_parts, axis=1)         # [128, NBLK*DEG*PACK]

        in_maps.append(
            {
                "inT": inT.astype(np.float32),
                "W_in": np.asarray(W, dtype=np.float32),
                "a2_in": a2,
                "bias_in": np.asarray(bias, dtype=np.float32),
                "idx16w": np.ascontiguousarray(idx16),
                "oneh_in": np.ascontiguousarray(oneh),
            }
        )
    return in_maps


def _reference_numpy(input_h, W, a, bias, indptr, indices):
    """Exact CPU fallback mirroring the jax reference (used only if the CSR is
    not the uniform-degree layout this kernel is specialized for)."""
    h = input_h.astype(np.float64) @ W.astype(np.float64) + bias.astype(np.float64)
    deg = np.diff(indptr.astype(np.int64))
    row = np.repeat(np.arange(N, dtype=np.int64), deg)
    e_cnt = indices.shape[0]
    if row.shape[0] < e_cnt:
        pad_val = row[-1] if row.shape[0] else 0
        row = np.pad(row, (0, e_cnt - row.shape[0]), constant_values=pad_val)
    row = row[:e_cnt]
    col = indices.astype(np.int64)
    a_src = a[:OUT_F, 0].astype(np.float64)
    a_dst = a[OUT_F:, 0].astype(np.float64)
    wh1 = h @ a_src
    wh2 = h @ a_dst
    e = wh1[row] + wh2[col]
    e = np.where(e >= 0, e, ALPHA * e)
    emax = np.full(N, -np.inf)
    np.maximum.at(emax, row, e)
    ex = np.exp(e - emax[row])
    den = np.zeros(N)
    np.add.at(den, row, ex)
    attn = ex / (den[row] + EPS)
    out = np.zeros((N, OUT_F))
    np.add.at(out, row, attn[:, None] * h[col])
    return out.astype(np.float32)


def kernel(input_h, W, a, bias, indptr, indices):
    input_h = np.asarray(input_h, dtype=np.float32)
    W = np.asarray(W, dtype=np.float32)
    a = np.asarray(a, dtype=np.float32)
    bias = np.asarray(bias, dtype=np.float32)
    indptr = np.asarray(indptr)
    indices_np = np.asarray(indices)

    expected_indptr = np.arange(N + 1, dtype=np.int64) * DEG
    if (
        indptr.shape[0] != N + 1
        or indices_np.shape[0] != E
        or not np.array_equal(indptr.astype(np.int64), expected_indptr)
    ):
        return _reference_numpy(input_h, W, a, bias, indptr, indices_np)

    _install_ntff_shim()
    _install_dma_gather_patch()
    from concourse.bass_utils import run_bass_kernel_spmd

    key = "gat"
    if key not in _PROGRAM_CACHE:
        _PROGRAM_CACHE[key] = build_program()
    nc = _PROGRAM_CACHE[key]

    in_maps = _host_prep(input_h, W, a, bias, indices_np)
    res = run_bass_kernel_spmd(nc, in_maps, core_ids=list(range(NCORES)))
    out = np.concatenate([res.results[c]["out_d"] for c in range(NCORES)], axis=0)
    return out.astype(np.float32)


if __name__ == "__main__":
    pass
